# revision 5
# baseline (speedup 1.0000x reference)
"""Chamfer loss kernel for Trainium2 (8 NeuronCores, batch-parallel).

Strategy
--------
dist2[m,n] = ||s_m||^2 - 2 s_m.d_n + ||d_n||^2 computed as a single K=16
augmented bf16 matmul per tile (hi/lo bf16 splits of coordinates and norms
keep ~2^-17 absolute accuracy; the PE runs bf16 at 1 cycle/row vs 4 for
fp32). Each core handles one batch. Per direction the PE produces the
4096x4096 dist2 matrix in [128 x 2048] PSUM tiles; the DVE reduces each
tile with a windowed min (W=16) giving [4096 rows x 256 windows] partial
minima. The host selects the best two windows per row, recomputes the
exact f32 distances for those 32 candidates, and finishes argmin, sigma
gather and the final means (cheap: 0.8% of the distance work).
"""

import numpy as np
import ml_dtypes

import concourse.bass as bass
import concourse.mybir as mybir
import concourse.tile as tile
from concourse.bass_utils import run_bass_kernel_spmd

BF16 = mybir.dt.bfloat16
F32 = mybir.dt.float32

B = 8
NPTS = 4096
KAUG = 16  # augmented contraction rows (15 used + 1 pad)
HALF = 2048  # columns per PSUM tile (4 banks); 2 halves per strip
W = 16  # min-window width
NWIN = HALF // W  # 128 windows per half-strip
NSTRIP = NPTS // 128  # 32 strips of 128 query rows
NHS = NSTRIP * 2  # 64 half-strips per direction

MAX_WAITS = 1  # walrus CoreV3 codegen rejects multiple sync waits per instruction


def _split_excess_waits(nc, max_waits=MAX_WAITS):
    """Move excess semaphore waits onto same-engine NoOps inserted right
    before the offending instruction (identical blocking semantics: the
    sequencer executes them in order)."""
    counter = [0]
    for bb in nc.main_func.blocks:
        insts = bb.instructions
        out = []
        for ins in insts:
            si = ins.sync_info
            waits = list(si.on_wait) if (si is not None and si.on_wait) else []
            if len(waits) > max_waits:
                extra = waits[: len(waits) - max_waits]
                si.on_wait = waits[len(waits) - max_waits :]
                for i in range(0, len(extra), max_waits):
                    counter[0] += 1
                    nop = mybir.InstNoOp(name=f"splitwait-{counter[0]}")
                    nop.engine = ins.engine
                    nop.sync_info = mybir.SyncInfo(
                        on_wait=extra[i : i + max_waits], on_update=[]
                    )
                    nc.register_instruction(nop)
                    out.append(nop)
            out.append(ins)
        insts[:] = out


def _build_nc():
    nc = bass.Bass()
    src_stat = nc.declare_dram_parameter("src_stat", [KAUG, NPTS], BF16, isOutput=False)
    dst_mov = nc.declare_dram_parameter("dst_mov", [KAUG, NPTS], BF16, isOutput=False)
    dst_stat = nc.declare_dram_parameter("dst_stat", [KAUG, NPTS], BF16, isOutput=False)
    src_mov = nc.declare_dram_parameter("src_mov", [KAUG, NPTS], BF16, isOutput=False)
    outf = nc.declare_dram_parameter("outf", [NHS, 128, NWIN], F32, isOutput=True)
    outb = nc.declare_dram_parameter("outb", [NHS, 128, NWIN], F32, isOutput=True)

    with tile.TileContext(nc) as tc:
        with (
            tc.tile_pool(name="aug", bufs=1) as augp,
            tc.tile_pool(name="psum", bufs=2, space="PSUM") as psp,
            tc.tile_pool(name="red", bufs=4) as redp,
        ):
            a_src_stat = augp.tile([KAUG, NPTS], BF16, tag="ss")
            a_dst_mov = augp.tile([KAUG, NPTS], BF16, tag="dm")
            a_dst_stat = augp.tile([KAUG, NPTS], BF16, tag="ds")
            a_src_mov = augp.tile([KAUG, NPTS], BF16, tag="sm")
            nc.sync.dma_start(a_src_stat[:], src_stat[:])
            nc.sync.dma_start(a_dst_mov[:], dst_mov[:])
            nc.sync.dma_start(a_dst_stat[:], dst_stat[:])
            nc.sync.dma_start(a_src_mov[:], src_mov[:])

            for stat, mov, outd in (
                (a_src_stat, a_dst_mov, outf),
                (a_dst_stat, a_src_mov, outb),
            ):
                for hs in range(NHS):
                    strip, half = divmod(hs, 2)
                    pt = psp.tile([128, HALF], F32, tag="pt")
                    for j in range(HALF // 512):
                        col = half * HALF + j * 512
                        nc.tensor.matmul(
                            pt[:, j * 512 : (j + 1) * 512],
                            stat[:, strip * 128 : (strip + 1) * 128],
                            mov[:, col : col + 512],
                            start=True,
                            stop=True,
                        )
                    rt = redp.tile([128, NWIN], F32, tag="rt")
                    nc.vector.tensor_reduce(
                        rt[:],
                        pt[:].rearrange("p (w c) -> p w c", c=W),
                        axis=mybir.AxisListType.X,
                        op=mybir.AluOpType.min,
                    )
                    nc.sync.dma_start(outd[hs], rt[:])
    _split_excess_waits(nc)
    return nc


def _split3(v):
    """Split f32 vector into three bf16 components summing to ~2^-26 rel."""
    h = v.astype(ml_dtypes.bfloat16)
    r = v - h.astype(np.float32)
    m = r.astype(ml_dtypes.bfloat16)
    l = (r - m.astype(np.float32)).astype(ml_dtypes.bfloat16)
    return h, m, l


def _aug_pair(x):
    """Build (stationary, moving) augmented matrices for points x [3, N]."""
    x = x.astype(np.float32)
    xh = x.astype(ml_dtypes.bfloat16)
    xl = (x - xh.astype(np.float32)).astype(ml_dtypes.bfloat16)
    n2 = (x * x).sum(axis=0, dtype=np.float32)
    nh, nm, nl = _split3(n2)
    npts = x.shape[1]
    ones = np.ones(npts, dtype=ml_dtypes.bfloat16)
    zero = np.zeros(npts, dtype=ml_dtypes.bfloat16)

    stat = np.stack(
        [xh[0], xh[1], xh[2], xl[0], xl[1], xl[2], xh[0], xh[1], xh[2],
         nh, nm, nl, ones, ones, ones, zero]
    )
    n2yh = (-2.0 * xh.astype(np.float32)).astype(ml_dtypes.bfloat16)
    n2yl = (-2.0 * xl.astype(np.float32)).astype(ml_dtypes.bfloat16)
    mov = np.stack(
        [n2yh[0], n2yh[1], n2yh[2], n2yh[0], n2yh[1], n2yh[2],
         n2yl[0], n2yl[1], n2yl[2], ones, ones, ones, nh, nm, nl, zero]
    )
    return stat, mov


def _unscramble(out):
    """[NHS, 128, NWIN] device layout -> [4096 rows, 256 windows]."""
    return (
        out.reshape(NSTRIP, 2, 128, NWIN)
        .transpose(0, 2, 1, 3)
        .reshape(NPTS, 2 * NWIN)
    )


def _refine(partials, x, y):
    """Exact min dist + argmin from windowed partial minima.

    partials: [Q, 256] approx window minima of dist2 for queries x [3, Q]
    against targets y [3, T]. Returns (min_dist [Q] f32, argmin [Q] int).
    """
    q = partials.shape[0]
    two = np.argpartition(partials, 1, axis=1)[:, :2]
    two = np.sort(two, axis=1)
    cols = (two[:, :, None] * W + np.arange(W)[None, None, :]).reshape(q, 2 * W)
    cand = y[:, cols]  # [3, Q, 2W]
    diff = cand - x[:, :, None]
    d2 = np.square(diff).sum(axis=0, dtype=np.float32)
    j = np.argmin(d2, axis=1)
    rows = np.arange(q)
    return np.sqrt(d2[rows, j]), cols[rows, j]


_NC_CACHE = []


def _get_nc():
    if not _NC_CACHE:
        _NC_CACHE.append(_build_nc())
    return _NC_CACHE[0]


def _run(in_maps, trace=False):
    nc = _get_nc()
    res = run_bass_kernel_spmd(nc, in_maps, list(range(B)), trace=trace)
    return res


def _make_in_maps(pc_src, pc_dst):
    in_maps = []
    for b in range(B):
        ss, sm = _aug_pair(pc_src[b])
        ds, dm = _aug_pair(pc_dst[b])
        in_maps.append(
            {"src_stat": ss, "dst_mov": dm, "dst_stat": ds, "src_mov": sm}
        )
    return in_maps


def _postprocess(results, pc_src, pc_dst, sigma_src, sigma_dst):
    fwd_terms = np.empty((B, NPTS), dtype=np.float32)
    bwd_terms = np.empty((B, NPTS), dtype=np.float32)
    for b in range(B):
        s = pc_src[b].astype(np.float32)
        d = pc_dst[b].astype(np.float32)
        pf = _unscramble(results[b]["outf"])
        pb = _unscramble(results[b]["outb"])
        fmin, fidx = _refine(pf, s, d)
        bmin, bidx = _refine(pb, d, s)
        fwd_terms[b] = fmin * (sigma_src[b] + sigma_dst[b][fidx]) * np.float32(0.5)
        bwd_terms[b] = bmin * (sigma_dst[b] + sigma_src[b][bidx]) * np.float32(0.5)
    loss = np.float32(fwd_terms.mean(dtype=np.float32)) + np.float32(
        bwd_terms.mean(dtype=np.float32)
    )
    return np.asarray(loss, dtype=np.float32)


def kernel(pc_src, pc_dst, sigma_src, sigma_dst):
    pc_src = np.asarray(pc_src, dtype=np.float32)
    pc_dst = np.asarray(pc_dst, dtype=np.float32)
    sigma_src = np.asarray(sigma_src, dtype=np.float32)
    sigma_dst = np.asarray(sigma_dst, dtype=np.float32)
    in_maps = _make_in_maps(pc_src, pc_dst)
    res = _run(in_maps, trace=False)
    return _postprocess(res.results, pc_src, pc_dst, sigma_src, sigma_dst)
